# revision 14
# baseline (speedup 1.0000x reference)
"""Trainium2 Bass kernel for the ConvFeatureExtractor problem.

Reference computation (all f32):
    matches[f, i] = sum_j kmer_params[f, kmer_idcs[i, j], j]      # (F, M)
    probs = softmax(matches / temperature, axis=1)                # over M
    pooled = freq @ probs.T                                       # (B, F)
    profile = pooled / pooled.sum(axis=1, keepdims=True)

Shapes: B=1024, M=4096 (=4^6 kmers), F=8192 filters, K=6, 4 bases.

Kernel strategy (8 NeuronCores, filter-sharded: FL = F/8 = 1024 per core):
  * matches^T = onehot(M, 24) @ params_flat^T(24, FL), where onehot
    one-hot-encodes kmer_idcs (built on host from the int32 index input).
    The 24-row contraction uses only a quarter of the PE array, so two
    k-tiles are packed into row bands 0-31 / 32-63 via tile_position and
    run concurrently (oh2/par2 hold the band-replicated data).
  * E = exp(matches/T) unnormalized (softmax denominator deferred).
  * 8 sub-passes over (fc chunk of 512, batch-tile pair), interleaved so
    the PE never idles: per k-tile inside an E sub-pass, one E-pair is
    emitted a step ahead (its ACT exp overlaps the U-matmuls) plus the
    U-matmuls U[b] = freq @ E^T accumulating in per-(b,fc) PSUM banks.
  * Batch tiles b6,b7 go FIRST: their rowsums complete early, so the
    AllReduce of s = rowsum(pooled) is split in four (b6,b7 / b0,b1 /
    b2,b3 / b4,b5); the first three hide under remaining matmuls and
    only the last 1KB AllReduce is tail-exposed.
  * Z[f] = sum_i E[i, f] via a DVE accumulation chain + a ones-column
    matmul; the slow reciprocal runs on an SBUF copy so it never holds
    a PSUM bank hostage.
  * drain per (b, fc): pooled = U * (1/Z) to SBUF + partial rowsum.
    (NOTE: fused tensor_tensor_reduce faults on this HW runtime —
    CoreSim passes but the NEFF dies with an NRT INTERNAL error.)
  * profile = pooled * (1/s) on ACT/DVE, DMA out per batch tile.
Each core returns its (B, FL) f32 slice; host concatenates along F.
"""

import os

import numpy as np
import ml_dtypes

import concourse.bass as bass  # noqa: F401  (AP types come through tile/bacc)
import concourse.tile as tile
from concourse import bacc, mybir
from concourse.bass_utils import run_bass_kernel_spmd

NCORES = 8
B = 1024           # batch
M = 4096           # 4^6 kmers
F = 8192           # filters
KMER = 6           # kmer length
NBASE = 4
KK = NBASE * KMER  # 24 flattened (base, position)
FL = F // NCORES   # 1024 filters per core

MT = M // 128      # 32 contraction tiles
NQ = MT // 2       # 16 row-tiled pairs of contraction tiles
BT = B // 128      # 8 batch tiles
FC = 512           # psum free chunk
NFC = FL // FC     # 2

BF16 = mybir.dt.bfloat16
F32 = mybir.dt.float32
AFT = mybir.ActivationFunctionType
ALU = mybir.AluOpType

# sub-pass schedule: (fc, batch-tile pair, compute_E, psum tags).
# Tag rotation is arranged so a reused tag's previous drain always
# completes at least one sub-pass before the reuse.
GROUPS = (
    (0, (6, 7), True, ("pu0", "pu1")),
    (1, (6, 7), True, ("pu2", "pu3")),
    (0, (0, 1), False, ("pu4", "pu5")),
    (1, (0, 1), False, ("pu0", "pu1")),
    (0, (2, 3), False, ("pu2", "pu3")),
    (1, (2, 3), False, ("pu4", "pu5")),
    (0, (4, 5), False, ("pu0", "pu1")),
    (1, (4, 5), False, ("pu2", "pu3")),
)

_CACHE: dict = {}


def _body(tc, freqT, onehot2, params2, tempr, out):
    nc = tc.nc
    with (
        tc.tile_pool(name="res", bufs=1) as res,
        tc.tile_pool(name="pm", bufs=2, space="PSUM") as pm,
        tc.tile_pool(name="pu", bufs=1, space="PSUM") as pu,
        tc.tile_pool(name="dram", bufs=1, space="DRAM") as dram,
        tc.tile_pool(name="outp", bufs=2) as outp,
    ):
        # ---------- PE warm-up (emitted FIRST: no DMA-gated op may precede
        # the memsets in the DVE queue, or the warm-up itself starts late) --
        # The PE clock gate (HAM) keeps the array at 1.2 GHz until it sees
        # ~3.4us of sustained matmul activity, re-throttling after similar
        # idle; row-tiled matmuls don't register as busy, so each core's
        # clock would otherwise warm at a chaotically different time,
        # drifting the cores ~20us apart and stretching every AllReduce's
        # entry barrier.  Dummy matmuls on a zeroed tile keep the array busy
        # from ~1us until the real stream begins on every core alike.
        ones_bf = res.tile([128, 128], BF16)  # lhsT: partition-sum + broadcast
        nc.vector.memset(ones_bf[:], 1.0)
        warm_sb = res.tile([128, FC], BF16)
        nc.vector.memset(warm_sb[:], 0.0)
        zacc = res.tile([128, FL], F32)
        nc.vector.memset(zacc[:], 0.0)
        n_warm = int(os.environ.get("KERNEL_WARM_MMS", "40"))
        for w in range(n_warm):
            wps = pm.tile([128, FC], F32, tag="pm", name=f"warm{w}")
            nc.tensor.matmul(wps[:], lhsT=ones_bf[:], rhs=warm_sb[:],
                             start=True, stop=True)

        # ---------- small inputs / constants ----------
        oh_sb = res.tile([56, M // 2], BF16)    # two 32-row bands of onehot^T
        nc.sync.dma_start(oh_sb[:], onehot2[:])
        par_sb = res.tile([56, FL], BF16)       # params^T replicated per band
        nc.sync.dma_start(par_sb[:], params2[:])
        t_sb = res.tile([128, 1], F32)       # T replicated on host to (128,1)
        nc.sync.dma_start(t_sb[:], tempr[:])
        invt_bc = res.tile([128, 1], F32)    # per-partition 1/T activation scale
        nc.vector.reciprocal(invt_bc[:], t_sb[:])

        # ---------- stream in freq^T (M, B), batch-pair major ----------
        freq_sb = res.tile([128, MT * B], BF16)
        for lo, hi in ((768, 1024), (0, 256), (256, 512), (512, 768)):
            for k in range(MT):
                nc.sync.dma_start(
                    freq_sb[:, k * B + lo: k * B + hi],
                    freqT[k * 128:(k + 1) * 128, lo:hi])

        E_sb = res.tile([128, MT * FL], BF16)
        U_sb = res.tile([128, BT * FL], F32)
        zacc_bf = res.tile([128, FL], BF16)
        zsb = res.tile([128, FL], F32)       # Z broadcast, SBUF copy
        invz_bc = res.tile([128, FL], F32)
        s_p0 = res.tile([128, BT], F32)      # fc0 partial rowsums
        s_col = res.tile([128, BT], F32)     # full per-core rowsums
        s_sum = res.tile([128, BT], F32)     # global rowsums (post-allreduce)
        rinv = res.tile([128, BT], F32)

        def e_pair(fc, q):
            # two row-tiled concurrent matmuls: k = 2q (band 0), 2q+1 (band 1)
            for j in (0, 1):
                k = 2 * q + j
                esl = slice(k * FL + fc * FC, k * FL + (fc + 1) * FC)
                pm_t = pm.tile([128, FC], F32, tag="pm", name=f"pm_{fc}_{k}")
                nc.tensor.matmul(pm_t[:],
                                 lhsT=oh_sb[32 * j:32 * j + KK,
                                            q * 128:(q + 1) * 128],
                                 rhs=par_sb[32 * j:32 * j + KK,
                                            fc * FC:(fc + 1) * FC],
                                 start=True, stop=True,
                                 tile_position=(32 * j, 0))
                nc.scalar.activation(E_sb[:, esl], pm_t[:], AFT.Exp,
                                     scale=invt_bc[:])
                nc.vector.tensor_add(zacc[:, fc * FC:(fc + 1) * FC],
                                     zacc[:, fc * FC:(fc + 1) * FC],
                                     E_sb[:, esl])

        def z_finish(fc):
            # ones(128,128).T @ zacc_bf = column sums broadcast to every
            # partition; copy PSUM->SBUF fast on ACT (frees the bank), then
            # the slow reciprocal runs out of SBUF off the critical path
            sl = slice(fc * FC, (fc + 1) * FC)
            nc.scalar.copy(zacc_bf[:, sl], zacc[:, sl])
            zps = pm.tile([128, FC], F32, tag="pm", name=f"zps{fc}")
            nc.tensor.matmul(zps[:], lhsT=ones_bf[:], rhs=zacc_bf[:, sl],
                             start=True, stop=True)
            nc.scalar.copy(zsb[:, sl], zps[:])
            nc.vector.reciprocal(invz_bc[:, sl], zsb[:, sl])

        def drain(bs, fc, pu_t):
            # pooled chunk = psum * invz to SBUF, then partial rowsum
            for j, b in enumerate(bs):
                dst = U_sb[:, b * FL + fc * FC: b * FL + (fc + 1) * FC]
                izl = invz_bc[:, fc * FC:(fc + 1) * FC]
                acc = (s_p0 if fc == 0 else s_col)[:, b:b + 1]
                nc.vector.tensor_mul(dst, pu_t[j][:], izl)
                nc.vector.reduce_sum(acc, dst, axis=mybir.AxisListType.X)
                if fc == 1:
                    nc.vector.tensor_add(acc, acc, s_p0[:, b:b + 1])

        no_coll = bool(os.environ.get("KERNEL_NO_COLLECTIVE"))

        def launch_allreduce(part, cols, ncols):
            if no_coll:
                nc.vector.tensor_scalar_mul(s_sum[:, cols], s_col[:, cols],
                                            float(NCORES))
            else:
                s_in = dram.tile([128, ncols], F32, name=f"sin{part}")
                s_out = dram.tile([128, ncols], F32, addr_space="Shared",
                                  name=f"sout{part}")
                nc.sync.dma_start(s_in[:], s_col[:, cols])
                nc.gpsimd.collective_compute(
                    "AllReduce", ALU.add,
                    replica_groups=[list(range(NCORES))],
                    ins=[s_in.opt()], outs=[s_out.opt()])
                nc.sync.dma_start(s_sum[:, cols], s_out[:])
            nc.vector.reciprocal(rinv[:, cols], s_sum[:, cols])

        def writeout(b, eng):
            prof = outp.tile([128, FL], F32, tag="prof", name=f"prof{b}")
            src = U_sb[:, b * FL:(b + 1) * FL]
            if eng == "act":
                nc.scalar.mul(prof[:], src, rinv[:, b:b + 1])
            else:
                nc.vector.tensor_scalar_mul(prof[:], src, rinv[:, b:b + 1])
            nc.sync.dma_start(out[b * 128:(b + 1) * 128, :], prof[:])

        # ---------- main: 8 sub-passes ----------
        sp0_drain = None
        for sp, (fc, bs, compute_E, tags) in enumerate(GROUPS):
            pu_t = [pu.tile([128, FC], F32, tag=tags[j], name=f"pu_{sp}_{j}")
                    for j in range(len(bs))]
            if compute_E:
                e_pair(fc, 0)
            for k in range(MT):
                if compute_E and k % 2 == 0 and k // 2 + 1 < NQ:
                    e_pair(fc, k // 2 + 1)
                rsl = slice(k * FL + fc * FC, k * FL + (fc + 1) * FC)
                for j, b in enumerate(bs):
                    nc.tensor.matmul(
                        pu_t[j][:],
                        lhsT=freq_sb[:, k * B + b * 128: k * B + (b + 1) * 128],
                        rhs=E_sb[:, rsl],
                        start=(k == 0), stop=(k == MT - 1))
            if compute_E:
                z_finish(fc)
            # sp0's drain needs invz0 (a slow reciprocal): defer it past
            # sp1's zacc chain so the chain isn't pushed past sp1's end
            if sp == 0:
                sp0_drain = (bs, fc, pu_t)
                continue
            if sp == 1:
                drain(*sp0_drain)
            drain(bs, fc, pu_t)
            # allreduces spaced > one mesh-latency apart so none queues
            # behind the previous on the collective cores
            if sp == 3:
                launch_allreduce(0, slice(6, 8), 2)
            if sp == 5:
                launch_allreduce(1, slice(0, 4), 4)
                writeout(6, "act")
                writeout(7, "act")
            if sp == 6 and os.environ.get("KERNEL_PRIME_AR") and not no_coll:
                # keep the collective firmware's hot loop spinning so the
                # final allreduce skips the ~11us ncfw wakeup
                p_in = dram.tile([128, 2], F32, name="prime_in")
                p_out = dram.tile([128, 2], F32, addr_space="Shared",
                                  name="prime_out")
                nc.sync.dma_start(p_in[:], s_col[:, 0:2])
                nc.gpsimd.collective_compute(
                    "AllReduce", ALU.add,
                    replica_groups=[list(range(NCORES))],
                    ins=[p_in.opt()], outs=[p_out.opt()])
            if sp == 7:
                launch_allreduce(2, slice(4, 6), 2)
                writeout(0, "act")
                writeout(1, "act")
                writeout(2, "act")
                writeout(3, "act")
                # tail: b4,b5 wait on the last allreduce; split ACT/DVE
                writeout(4, "act")
                writeout(5, "vec")


def _build_bass():
    nc = bacc.Bacc("TRN2", target_bir_lowering=False, debug=False,
                   num_devices=NCORES)
    freqT = nc.dram_tensor("freqT", [M, B], BF16, kind="ExternalInput").ap()
    onehot2 = nc.dram_tensor("onehot2", [56, M // 2], BF16,
                             kind="ExternalInput").ap()
    params2 = nc.dram_tensor("params2", [56, FL], BF16,
                             kind="ExternalInput").ap()
    tempr = nc.dram_tensor("tempr", [128, 1], F32, kind="ExternalInput").ap()
    out = nc.dram_tensor("out", [B, FL], F32, kind="ExternalOutput").ap()

    with tile.TileContext(nc) as tc:
        _body(tc, freqT, onehot2, params2, tempr, out)
    nc.compile()
    return nc


def _get_nc():
    if "nc" not in _CACHE:
        _CACHE["nc"] = _build_bass()
    return _CACHE["nc"]


def _prepare_in_maps(freq, kmer_params, temperature, kmer_idcs):
    freq = np.asarray(freq, dtype=np.float32)            # (B, M)
    kp = np.asarray(kmer_params, dtype=np.float32)       # (F, 4, K)
    temp = np.asarray(temperature, dtype=np.float32).reshape(-1)[:1]
    idcs = np.asarray(kmer_idcs).astype(np.int64)        # (M, K)

    assert freq.shape == (B, M) and kp.shape == (F, NBASE, KMER)
    assert idcs.shape == (M, KMER)

    # one-hot re-encoding of the index input: onehot[i, c*K + j] = 1 iff
    # kmer_idcs[i, j] == c   (params_flat[f, c*K + j] = kmer_params[f, c, j])
    onehot = np.zeros((M, NBASE, KMER), dtype=np.float32)
    onehot[np.arange(M)[:, None], idcs, np.arange(KMER)[None, :]] = 1.0
    onehotT = onehot.reshape(M, KK).T                    # (24, M)

    # two 32-row bands: band j holds k-tiles with k%2==j, pair-major cols
    onehot2 = np.zeros((56, M // 2), dtype=np.float32)
    ohT3 = onehotT.reshape(KK, MT, 128)                  # (24, 32, 128)
    for j in range(2):
        onehot2[32 * j:32 * j + KK] = (
            ohT3[:, j::2, :].reshape(KK, M // 2))
    onehot2 = np.ascontiguousarray(onehot2).astype(ml_dtypes.bfloat16)

    params_flat = kp.reshape(F, KK)
    freqT = np.ascontiguousarray(freq.T).astype(ml_dtypes.bfloat16)
    tempr = np.ascontiguousarray(np.broadcast_to(temp.reshape(1, 1), (128, 1)))

    in_maps = []
    for c in range(NCORES):
        parT_c = params_flat[c * FL:(c + 1) * FL].T      # (24, FL)
        params2 = np.zeros((56, FL), dtype=np.float32)
        params2[0:KK] = parT_c
        params2[32:32 + KK] = parT_c
        in_maps.append({
            "freqT": freqT,
            "onehot2": onehot2,
            "params2": np.ascontiguousarray(params2).astype(ml_dtypes.bfloat16),
            "tempr": tempr,
        })
    return in_maps


def _run(in_maps, trace=False):
    nc = _get_nc()
    return run_bass_kernel_spmd(nc, in_maps, list(range(NCORES)), trace=trace)


def kernel(freq, kmer_params, temperature, kmer_idcs):
    in_maps = _prepare_in_maps(freq, kmer_params, temperature, kmer_idcs)
    res = _run(in_maps,
               trace=os.environ.get("KERNEL_TRACE", "") not in ("", "0"))
    _CACHE["last_result"] = res
    return np.concatenate(
        [np.asarray(res.results[c]["out"], dtype=np.float32)
         for c in range(NCORES)], axis=1)


# revision 15
# speedup vs baseline: 1.0748x; 1.0748x over previous
"""Trainium2 Bass kernel for the ConvFeatureExtractor problem.

Reference computation (all f32):
    matches[f, i] = sum_j kmer_params[f, kmer_idcs[i, j], j]      # (F, M)
    probs = softmax(matches / temperature, axis=1)                # over M
    pooled = freq @ probs.T                                       # (B, F)
    profile = pooled / pooled.sum(axis=1, keepdims=True)

Shapes: B=1024, M=4096 (=4^6 kmers), F=8192 filters, K=6, 4 bases.

Kernel strategy (8 NeuronCores, filter-sharded: FL = F/8 = 1024 per core):
  * matches^T = onehot(M, 24) @ params_flat^T(24, FL) as K=24 matmuls,
    where onehot one-hot-encodes kmer_idcs (built on host from the int32
    index input; it is a pure re-encoding of that input).  Full-row
    matmuls only: row-tiled (tile_position) packing halves the E cost on
    paper but does not register as "busy" with the PE clock gate (HAM),
    which then throttles the whole phase to 1.2 GHz and desyncs the
    cores — measured strictly slower.
  * E = exp(matches/T) unnormalized (softmax denominator deferred).
  * A PE warm-up block of dummy matmuls runs from ~9us (earliest the
    engines wake) until the first input-gated matmul, so HAM reaches
    2.4 GHz before the real stream and every core warms identically
    (core skew otherwise inflates every AllReduce's entry wait).
  * 8 sub-passes over (fc chunk of 512, batch-tile group), interleaved
    so the PE never idles: in E sub-passes the E-matmul for k+1 is
    emitted before the U-matmuls of k (U[b] = freq @ E^T accumulating
    in per-(b,fc) PSUM banks), hiding the ACT exp under PE work.
  * Batch tiles 5,6,7 go FIRST and tile 4 goes LAST ALONE: the
    AllReduce of s = rowsum(pooled) is split (b5..7 / b0..3 / b4); the
    first two hide under remaining matmuls and the tail pays only one
    ~0.5KB AllReduce plus a single writeout.
  * Z[f] = sum_i E[i, f] via a DVE accumulation chain + a ones-column
    matmul; the slow reciprocal runs on an SBUF copy so it never holds
    a PSUM bank hostage.
  * drain per (b, fc): pooled = U * (1/Z) to SBUF + partial rowsum.
    (NOTE: fused tensor_tensor_reduce faults on this HW runtime —
    CoreSim passes but the NEFF dies with an NRT INTERNAL error.)
  * profile = pooled * (1/s) on ACT/DVE, DMA out per batch tile.
Each core returns its (B, FL) f32 slice; host concatenates along F.
"""

import os

import numpy as np
import ml_dtypes

import concourse.bass as bass  # noqa: F401  (AP types come through tile/bacc)
import concourse.tile as tile
from concourse import bacc, mybir
from concourse.bass_utils import run_bass_kernel_spmd

NCORES = 8
B = 1024           # batch
M = 4096           # 4^6 kmers
F = 8192           # filters
KMER = 6           # kmer length
NBASE = 4
KK = NBASE * KMER  # 24 flattened (base, position)
FL = F // NCORES   # 1024 filters per core

MT = M // 128      # 32 contraction tiles
BT = B // 128      # 8 batch tiles
FC = 512           # psum free chunk
NFC = FL // FC     # 2

BF16 = mybir.dt.bfloat16
F32 = mybir.dt.float32
AFT = mybir.ActivationFunctionType
ALU = mybir.AluOpType

# sub-pass schedule: (fc, batch-tile group, compute_E, psum tags).
# Tag rotation is arranged so a reused tag's previous drain always
# completes well before the reuse (drains of E sub-passes land ~8us
# into the following sub-pass; each sub-pass is >=8.4us long).
GROUPS = (
    (0, (5, 6, 7), True, ("pu0", "pu1", "pu2")),
    (1, (5, 6, 7), True, ("pu3", "pu4", "pu5")),
    (0, (0, 1), False, ("pu0", "pu1")),
    (1, (0, 1), False, ("pu2", "pu3")),
    (0, (2, 3), False, ("pu4", "pu5")),
    (1, (2, 3), False, ("pu0", "pu1")),
    (0, (4,), False, ("pu2",)),
    (1, (4,), False, ("pu3",)),
)

_CACHE: dict = {}


def _body(tc, freqT, onehotT, paramsT, tempr, out):
    nc = tc.nc
    with (
        tc.tile_pool(name="res", bufs=1) as res,
        tc.tile_pool(name="pm", bufs=2, space="PSUM") as pm,
        tc.tile_pool(name="pu", bufs=1, space="PSUM") as pu,
        tc.tile_pool(name="dram", bufs=1, space="DRAM") as dram,
        tc.tile_pool(name="outp", bufs=2) as outp,
    ):
        # ---------- PE warm-up (emitted FIRST: no DMA-gated op may precede
        # the memsets in the DVE queue, or the warm-up itself starts late) --
        ones_bf = res.tile([128, 128], BF16)  # lhsT: partition-sum + broadcast
        nc.vector.memset(ones_bf[:], 1.0)
        warm_sb = res.tile([128, FC], BF16)
        nc.vector.memset(warm_sb[:], 0.0)
        zacc = res.tile([128, FL], F32)
        nc.vector.memset(zacc[:], 0.0)
        n_warm = int(os.environ.get("KERNEL_WARM_MMS", "14"))
        for w in range(n_warm):
            wps = pm.tile([128, FC], F32, tag="pm", name=f"warm{w}")
            nc.tensor.matmul(wps[:], lhsT=ones_bf[:], rhs=warm_sb[:],
                             start=True, stop=True)

        # ---------- small inputs / constants ----------
        oh_sb = res.tile([KK, M], BF16)
        nc.sync.dma_start(oh_sb[:], onehotT[:])
        par_sb = res.tile([KK, FL], BF16)
        nc.sync.dma_start(par_sb[:], paramsT[:])
        t_sb = res.tile([128, 1], F32)       # T replicated on host to (128,1)
        nc.sync.dma_start(t_sb[:], tempr[:])
        invt_bc = res.tile([128, 1], F32)    # per-partition 1/T activation scale
        nc.vector.reciprocal(invt_bc[:], t_sb[:])

        # ---------- stream in freq^T (M, B), batch-group major ----------
        freq_sb = res.tile([128, MT * B], BF16)
        for lo, hi in ((640, 1024), (0, 256), (256, 512), (512, 640)):
            for k in range(MT):
                nc.sync.dma_start(
                    freq_sb[:, k * B + lo: k * B + hi],
                    freqT[k * 128:(k + 1) * 128, lo:hi])

        E_sb = res.tile([128, MT * FL], BF16)
        U_sb = res.tile([128, BT * FL], F32)
        zacc_bf = res.tile([128, FL], BF16)
        zsb = res.tile([128, FL], F32)       # Z broadcast, SBUF copy
        invz_bc = res.tile([128, FL], F32)
        s_p0 = res.tile([128, BT], F32)      # fc0 partial rowsums
        s_col = res.tile([128, BT], F32)     # full per-core rowsums
        s_sum = res.tile([128, BT], F32)     # global rowsums (post-allreduce)
        rinv = res.tile([128, BT], F32)

        def e_mm(fc, k):
            # matches chunk -> exp -> E_sb; Z partial accumulation on DVE
            esl = slice(k * FL + fc * FC, k * FL + (fc + 1) * FC)
            pm_t = pm.tile([128, FC], F32, tag="pm", name=f"pm_{fc}_{k}")
            nc.tensor.matmul(pm_t[:],
                             lhsT=oh_sb[:, k * 128:(k + 1) * 128],
                             rhs=par_sb[:, fc * FC:(fc + 1) * FC],
                             start=True, stop=True)
            nc.scalar.activation(E_sb[:, esl], pm_t[:], AFT.Exp,
                                 scale=invt_bc[:])
            nc.vector.tensor_add(zacc[:, fc * FC:(fc + 1) * FC],
                                 zacc[:, fc * FC:(fc + 1) * FC],
                                 E_sb[:, esl])

        def z_finish(fc):
            # ones(128,128).T @ zacc_bf = column sums broadcast to every
            # partition; copy PSUM->SBUF fast on ACT (frees the bank), then
            # the slow reciprocal runs out of SBUF off the critical path
            sl = slice(fc * FC, (fc + 1) * FC)
            nc.scalar.copy(zacc_bf[:, sl], zacc[:, sl])
            zps = pm.tile([128, FC], F32, tag="pm", name=f"zps{fc}")
            nc.tensor.matmul(zps[:], lhsT=ones_bf[:], rhs=zacc_bf[:, sl],
                             start=True, stop=True)
            nc.scalar.copy(zsb[:, sl], zps[:])
            nc.vector.reciprocal(invz_bc[:, sl], zsb[:, sl])

        def drain(bs, fc, pu_t):
            # pooled chunk = psum * invz to SBUF, then partial rowsum
            for j, b in enumerate(bs):
                dst = U_sb[:, b * FL + fc * FC: b * FL + (fc + 1) * FC]
                izl = invz_bc[:, fc * FC:(fc + 1) * FC]
                acc = (s_p0 if fc == 0 else s_col)[:, b:b + 1]
                nc.vector.tensor_mul(dst, pu_t[j][:], izl)
                nc.vector.reduce_sum(acc, dst, axis=mybir.AxisListType.X)
                if fc == 1:
                    nc.vector.tensor_add(acc, acc, s_p0[:, b:b + 1])

        no_coll = bool(os.environ.get("KERNEL_NO_COLLECTIVE"))

        def launch_allreduce(part, cols, ncols):
            if no_coll:
                nc.vector.tensor_scalar_mul(s_sum[:, cols], s_col[:, cols],
                                            float(NCORES))
            else:
                s_in = dram.tile([128, ncols], F32, name=f"sin{part}")
                s_out = dram.tile([128, ncols], F32, addr_space="Shared",
                                  name=f"sout{part}")
                nc.sync.dma_start(s_in[:], s_col[:, cols])
                nc.gpsimd.collective_compute(
                    "AllReduce", ALU.add,
                    replica_groups=[list(range(NCORES))],
                    ins=[s_in.opt()], outs=[s_out.opt()])
                nc.sync.dma_start(s_sum[:, cols], s_out[:])
            nc.vector.reciprocal(rinv[:, cols], s_sum[:, cols])

        def writeout(b, eng):
            prof = outp.tile([128, FL], F32, tag="prof", name=f"prof{b}")
            src = U_sb[:, b * FL:(b + 1) * FL]
            if eng == "act":
                nc.scalar.mul(prof[:], src, rinv[:, b:b + 1])
            else:
                nc.vector.tensor_scalar_mul(prof[:], src, rinv[:, b:b + 1])
            nc.sync.dma_start(out[b * 128:(b + 1) * 128, :], prof[:])

        # ---------- main: 8 sub-passes ----------
        for sp, (fc, bs, compute_E, tags) in enumerate(GROUPS):
            pu_t = [pu.tile([128, FC], F32, tag=tags[j], name=f"pu_{sp}_{j}")
                    for j in range(len(bs))]
            if compute_E:
                e_mm(fc, 0)
            for k in range(MT):
                if compute_E and k + 1 < MT:
                    e_mm(fc, k + 1)
                rsl = slice(k * FL + fc * FC, k * FL + (fc + 1) * FC)
                for j, b in enumerate(bs):
                    nc.tensor.matmul(
                        pu_t[j][:],
                        lhsT=freq_sb[:, k * B + b * 128: k * B + (b + 1) * 128],
                        rhs=E_sb[:, rsl],
                        start=(k == 0), stop=(k == MT - 1))
            if compute_E:
                z_finish(fc)
            drain(bs, fc, pu_t)
            # allreduces spaced > one mesh-latency apart so none queues
            # behind the previous on the collective cores
            if sp == 1:
                launch_allreduce(0, slice(5, 8), 3)
            if sp == 5:
                launch_allreduce(1, slice(0, 4), 4)
                writeout(5, "act")
                writeout(6, "act")
                writeout(7, "act")
            if sp == 6 and os.environ.get("KERNEL_PRIME_AR") and not no_coll:
                # keep the collective firmware's hot loop spinning so the
                # final allreduce skips part of the ncfw wakeup
                p_in = dram.tile([128, 2], F32, name="prime_in")
                p_out = dram.tile([128, 2], F32, addr_space="Shared",
                                  name="prime_out")
                nc.sync.dma_start(p_in[:], s_col[:, 0:2])
                nc.gpsimd.collective_compute(
                    "AllReduce", ALU.add,
                    replica_groups=[list(range(NCORES))],
                    ins=[p_in.opt()], outs=[p_out.opt()])
            if sp == 7:
                launch_allreduce(2, slice(4, 5), 1)
                writeout(0, "act")
                writeout(1, "act")
                writeout(2, "act")
                writeout(3, "vec")
                # tail: only b4 waits on the last (0.5KB) allreduce
                writeout(4, "act")


def _build_bass():
    nc = bacc.Bacc("TRN2", target_bir_lowering=False, debug=False,
                   num_devices=NCORES)
    freqT = nc.dram_tensor("freqT", [M, B], BF16, kind="ExternalInput").ap()
    onehotT = nc.dram_tensor("onehotT", [KK, M], BF16, kind="ExternalInput").ap()
    paramsT = nc.dram_tensor("paramsT", [KK, FL], BF16, kind="ExternalInput").ap()
    tempr = nc.dram_tensor("tempr", [128, 1], F32, kind="ExternalInput").ap()
    out = nc.dram_tensor("out", [B, FL], F32, kind="ExternalOutput").ap()

    with tile.TileContext(nc) as tc:
        _body(tc, freqT, onehotT, paramsT, tempr, out)
    nc.compile()
    return nc


def _get_nc():
    if "nc" not in _CACHE:
        _CACHE["nc"] = _build_bass()
    return _CACHE["nc"]


def _prepare_in_maps(freq, kmer_params, temperature, kmer_idcs):
    freq = np.asarray(freq, dtype=np.float32)            # (B, M)
    kp = np.asarray(kmer_params, dtype=np.float32)       # (F, 4, K)
    temp = np.asarray(temperature, dtype=np.float32).reshape(-1)[:1]
    idcs = np.asarray(kmer_idcs).astype(np.int64)        # (M, K)

    assert freq.shape == (B, M) and kp.shape == (F, NBASE, KMER)
    assert idcs.shape == (M, KMER)

    # one-hot re-encoding of the index input: onehot[i, c*K + j] = 1 iff
    # kmer_idcs[i, j] == c   (params_flat[f, c*K + j] = kmer_params[f, c, j])
    onehot = np.zeros((M, NBASE, KMER), dtype=np.float32)
    onehot[np.arange(M)[:, None], idcs, np.arange(KMER)[None, :]] = 1.0
    onehotT = np.ascontiguousarray(
        onehot.reshape(M, KK).T).astype(ml_dtypes.bfloat16)

    params_flat = kp.reshape(F, KK)
    freqT = np.ascontiguousarray(freq.T).astype(ml_dtypes.bfloat16)
    tempr = np.ascontiguousarray(np.broadcast_to(temp.reshape(1, 1), (128, 1)))

    in_maps = []
    for c in range(NCORES):
        paramsT_c = np.ascontiguousarray(
            params_flat[c * FL:(c + 1) * FL].T).astype(ml_dtypes.bfloat16)
        in_maps.append({
            "freqT": freqT,
            "onehotT": onehotT,
            "paramsT": paramsT_c,
            "tempr": tempr,
        })
    return in_maps


def _run(in_maps, trace=False):
    nc = _get_nc()
    return run_bass_kernel_spmd(nc, in_maps, list(range(NCORES)), trace=trace)


def kernel(freq, kmer_params, temperature, kmer_idcs):
    in_maps = _prepare_in_maps(freq, kmer_params, temperature, kmer_idcs)
    res = _run(in_maps,
               trace=os.environ.get("KERNEL_TRACE", "") not in ("", "0"))
    _CACHE["last_result"] = res
    return np.concatenate(
        [np.asarray(res.results[c]["out"], dtype=np.float32)
         for c in range(NCORES)], axis=1)
